# revision 17
# baseline (speedup 1.0000x reference)
"""Distributed causal attention w/ RoPE for TRN2 (8 NeuronCores).

Sharding: 2-way data-parallel over batch x 4-way tensor-parallel over
heads. Cores {0..3} handle batch 0, {4..7} batch 1; core rank i within
its group owns heads 4i..4i+3 (as two head-pairs packed into 128
partitions). Per core:
  - QKV projection of its batch's full sequence for its 4 heads, in
    transposed layout (qkv^T = W^T.T @ x^T); V is instead projected with
    x as the stationary operand (out[t,d] = x_chunk^T @ w_v), which
    yields V directly in [tk, d] layout -- no on-device V transpose.
  - RoPE via pair-swapped duplicate values (strided SBUF DMA) +
    elementwise DVE ops.
  - Causal attention per (query-group, head-pair): scores for both heads
    of a pair go into one 2-bank PSUM tile so a single merged Exp
    instruction covers them; AV matmuls are software-pipelined one tile
    ahead of the scores stream to keep the PE p-state high. V tiles carry
    64 extra all-ones columns so the AV matmul replicates the softmax
    denominator into PSUM rows 64:128 (a free partition-broadcast on the
    PE); normalization is recip+mul on DVE only.
  - Output projection partials reduced with one ReduceScatter per
    query-group across the 4-core batch group (the two groups' collectives
    are independent); each proj is emitted deferred, two attention steps
    into the next group, so the PE never waits on the normalize chain.
Host side: input layout prep and concatenation of ReduceScatter shards.
"""

import numpy as np

import concourse.bass as bass
import concourse.bacc as bacc
import concourse.mybir as mybir
from concourse import tile
from concourse.bass_utils import run_bass_kernel_spmd

B, T, C, H, D = 2, 2048, 1024, 16, 64
NCORE = 8
GRP = 4                   # cores per batch group
HPC = H // GRP            # heads per core = 4
NPAIR = HPC // 2          # head-pairs per core = 2
TCH = 512                 # token chunk (qkv proj free dim & query group)
NTC = T // TCH            # 4
NBLK = T // 128           # 16 tk tiles
VW = 128                  # vaug block: 64 V cols + 64 ones cols
ROPE_BASE = 10000.0
F32 = mybir.dt.float32
F16 = mybir.dt.float16


def _rope_tables():
    # row p of a pair-tile holds head_local = p // 64, d = p % 64
    d = np.arange(D)
    j = d // 2
    theta = ROPE_BASE ** (-(2.0 * j) / D)          # per-row theta
    t = np.arange(T, dtype=np.float64)
    ang = t[None, :] * theta[:, None]              # [64, T]
    cos = np.cos(ang)
    sin = np.sin(ang)
    sgn = np.where(d % 2 == 0, -1.0, 1.0)[:, None]
    c1 = np.concatenate([cos, cos], axis=0)        # [128, T]
    s1 = np.concatenate([sgn * sin, sgn * sin], axis=0)
    scale = 1.0 / np.sqrt(D)
    return (
        (c1 * scale).astype(np.float16),
        (s1 * scale).astype(np.float16),
        c1.astype(np.float16),
        s1.astype(np.float16),
    )


def build():
    nc = bacc.Bacc(num_devices=NCORE)
    x_t = nc.declare_dram_parameter("x_t", [C, T], F16, isOutput=False)
    w_all = nc.declare_dram_parameter("w_all", [C, 768], F16, isOutput=False)
    w_p = nc.declare_dram_parameter("w_p", [C, 256], F16, isOutput=False)
    out_ext = nc.declare_dram_parameter("out", [NTC, 256, TCH], F16,
                                        isOutput=True)

    cq_np, sq_np, ck_np, sk_np = _rope_tables()
    tk = np.arange(128)[:, None]
    tq = np.arange(128)[None, :]
    mask_np = (tq >= tk).astype(np.float16)
    cq_c = nc.inline_tensor(cq_np, name="cq")
    sq_c = nc.inline_tensor(sq_np, name="sq")
    ck_c = nc.inline_tensor(ck_np, name="ck")
    sk_c = nc.inline_tensor(sk_np, name="sk")
    mask_c = nc.inline_tensor(mask_np, name="mask")

    cc_in = [nc.dram_tensor(f"cc_in{j}", [256, TCH], F16) for j in range(NTC)]
    cc_out = [nc.dram_tensor(f"cc_out{j}", [C, TCH], F16)
              for j in range(NTC)]
    cc_half_in = [[nc.dram_tensor(f"cc_hin{j}_{p}", [128, TCH], F16)
                   for p in range(NPAIR)] for j in range(NTC)]
    cc_half_out = [[nc.dram_tensor(f"cc_hout{j}_{p}", [512, TCH], F16)
                    for p in range(NPAIR)] for j in range(NTC)]
    cc_win = nc.dram_tensor("cc_win", [1, 128], F16)
    cc_wout = nc.dram_tensor("cc_wout", [GRP, 128], F16)
    groups = [[0, 1, 2, 3], [4, 5, 6, 7]]

    with tile.TileContext(nc) as tc:
        with (
            tc.tile_pool(name="const", bufs=1) as cpool,
            tc.tile_pool(name="big", bufs=1) as bpool,
            tc.tile_pool(name="xt", bufs=16) as xpool,
            tc.tile_pool(name="tmp", bufs=4) as tpool,
            tc.tile_pool(name="exp", bufs=4) as epool,
            tc.tile_pool(name="ysmall", bufs=2) as ypool_sb,
            tc.tile_pool(name="s2", bufs=2, space="PSUM") as s2pool,
            tc.tile_pool(name="ypsum", bufs=2, space="PSUM") as ypool,
            tc.tile_pool(name="mm", bufs=2, space="PSUM") as mmpool,
        ):
            # ---- persistent SBUF loads (weights first: unblock matmuls) ----
            w_sb = []
            for c in range(8):
                w = cpool.tile([128, 768], F16, tag=f"w{c}")
                nc.scalar.dma_start(out=w[:, :],
                                    in_=w_all[c * 128:(c + 1) * 128, :])
                w_sb.append(w)
            wp_sb = []
            for k in range(8):
                wp = cpool.tile([128, 256], F16, tag=f"wp{k}")
                nc.scalar.dma_start(out=wp[:, :],
                                    in_=w_p[k * 128:(k + 1) * 128, :])
                wp_sb.append(wp)
            cq_sb = cpool.tile([128, T], F16, tag="cq")
            nc.scalar.dma_start(out=cq_sb[:, :], in_=cq_c[:, :])
            sq_sb = cpool.tile([128, T], F16, tag="sq")
            nc.scalar.dma_start(out=sq_sb[:, :], in_=sq_c[:, :])
            ck_sb = cpool.tile([128, T], F16, tag="ck")
            nc.scalar.dma_start(out=ck_sb[:, :], in_=ck_c[:, :])
            sk_sb = cpool.tile([128, T], F16, tag="sk")
            nc.scalar.dma_start(out=sk_sb[:, :], in_=sk_c[:, :])
            mask_sb = cpool.tile([128, 128], F16, tag="mask")
            nc.scalar.dma_start(out=mask_sb[:, :], in_=mask_c[:, :])

            # warmup collective to absorb one-time mesh/channel setup
            nc.gpsimd.collective_compute(
                "AllGather", mybir.AluOpType.bypass,
                replica_groups=groups,
                ins=[cc_win.ap().opt()],
                outs=[cc_wout.ap().opt()])

            rope_q = [bpool.tile([128, T], F16, tag=f"rope_q{p}",
                                 name=f"rope_q{p}") for p in range(NPAIR)]
            rope_k = [bpool.tile([128, T], F16, tag=f"rope_k{p}",
                                 name=f"rope_k{p}") for p in range(NPAIR)]
            # one [tk, d|ones] V tile per head, packed in a single buffer:
            # head h occupies cols [h*NBLK*VW, (h+1)*NBLK*VW)
            vaug = bpool.tile([128, HPC * NBLK * VW], F16, tag="vaug")
            nc.vector.memset(vaug[:, :], 1.0)
            va3 = vaug[:, :].rearrange("p (h b) -> p h b", h=HPC)

            # ---- QKV projection + fused RoPE ------------------------------
            for tc_i in range(NTC):
                t0 = tc_i * TCH
                xts = []
                for c in range(8):
                    xt = xpool.tile([128, TCH], F16, tag="xt")
                    nc.sync.dma_start(
                        out=xt[:, :],
                        in_=x_t[c * 128:(c + 1) * 128, t0:t0 + TCH])
                    xts.append(xt)
                for p in range(NPAIR):
                    qk_ps = s2pool.tile([128, 1024], F32, tag="s2")
                    for m in range(2):  # q, k
                        for c in range(8):
                            nc.tensor.matmul(
                                qk_ps[:, m * 512:(m + 1) * 512],
                                w_sb[c][:, p * 256 + m * 128:
                                        p * 256 + (m + 1) * 128],
                                xts[c][:, :],
                                start=(c == 0), stop=(c == 7))
                    qk_sb = tpool.tile([128, 1024], F16, tag="qksb")
                    nc.vector.tensor_copy(qk_sb[:, :], qk_ps[:, :])
                    qks_sw = tpool.tile([128, 1024], F16, tag="qkssb")
                    nc.sync.dma_start(out=qks_sw[0::2, :], in_=qk_sb[1::2, :])
                    nc.sync.dma_start(out=qks_sw[1::2, :], in_=qk_sb[0::2, :])
                    # rope_q = q*cq + qs*sq ; rope_k = k*ck + ks*sk
                    tq1 = tpool.tile([128, TCH], F16, tag="tmp")
                    nc.vector.tensor_mul(tq1[:, :], qk_sb[:, 0:512],
                                         cq_sb[:, t0:t0 + TCH])
                    tq2 = tpool.tile([128, TCH], F16, tag="tmp")
                    nc.vector.tensor_mul(tq2[:, :], qks_sw[:, 0:512],
                                         sq_sb[:, t0:t0 + TCH])
                    nc.vector.tensor_add(rope_q[p][:, t0:t0 + TCH], tq1[:, :],
                                         tq2[:, :])
                    tk1 = tpool.tile([128, TCH], F16, tag="tmp")
                    nc.vector.tensor_mul(tk1[:, :], qk_sb[:, 512:1024],
                                         ck_sb[:, t0:t0 + TCH])
                    tk2 = tpool.tile([128, TCH], F16, tag="tmp")
                    nc.vector.tensor_mul(tk2[:, :], qks_sw[:, 512:1024],
                                         sk_sb[:, t0:t0 + TCH])
                    nc.vector.tensor_add(rope_k[p][:, t0:t0 + TCH], tk1[:, :],
                                         tk2[:, :])
                # V in [tk, d] layout directly: x chunks stationary
                for sub in range(4):
                    Tt = 4 * tc_i + sub
                    vt_ps = mmpool.tile([128, TCH], F32, tag="mm")
                    for c in range(8):
                        nc.tensor.matmul(
                            vt_ps[:, 0:256],
                            xts[c][:, sub * 128:(sub + 1) * 128],
                            w_sb[c][:, 512:768],
                            start=(c == 0), stop=(c == 7))
                    nc.vector.tensor_copy(
                        va3[:, :, Tt * VW:Tt * VW + 64],
                        vt_ps[:, 0:256].rearrange("p (h d) -> p h d", d=64))

            # ---- attention + deferred output projection -------------------
            pending = [None]

            def attn_set(g, p, flush_at=15):
                """Attention for query group g, head pair p; returns the
                normalized [128, TCH] fp16 y tile (rows = 2 heads x 64)."""
                ntk = 4 * g + 4
                base = p * ntk
                q0 = g * TCH

                def emit_scores(Tt):
                    diag = (Tt // 4 == g)
                    r = Tt % 4
                    c0 = 128 * r if diag else 0
                    s2 = s2pool.tile([128, 1024], F32, tag="s2")
                    for i in range(2):
                        nc.tensor.matmul(
                            s2[:, i * 512 + c0:(i + 1) * 512],
                            rope_k[p][i * 64:(i + 1) * 64,
                                      Tt * 128:(Tt + 1) * 128],
                            rope_q[p][i * 64:(i + 1) * 64,
                                      q0 + c0:q0 + TCH],
                            start=True, stop=True)
                    e2 = epool.tile([128, 1024], F16, tag="e2")
                    if diag:
                        for i in range(2):
                            nc.scalar.activation(
                                e2[:, i * 512 + c0:(i + 1) * 512],
                                s2[:, i * 512 + c0:(i + 1) * 512],
                                mybir.ActivationFunctionType.Exp)
                            nc.vector.tensor_mul(
                                e2[:, i * 512 + c0:i * 512 + c0 + 128],
                                e2[:, i * 512 + c0:i * 512 + c0 + 128],
                                mask_sb[:, :])
                    else:
                        nc.scalar.activation(
                            e2[:, :], s2[:, :],
                            mybir.ActivationFunctionType.Exp)
                    return e2, c0

                y_ps = [ypool.tile([128, TCH], F32, tag="y", name=f"y{i}")
                        for i in range(2)]
                cur = emit_scores(0)
                for Tt in range(ntk):
                    e2, c0 = cur
                    if Tt + 1 < ntk:
                        cur = emit_scores(Tt + 1)
                    if base + Tt >= flush_at and pending[0] is not None:
                        pending[0]()
                        pending[0] = None
                    for i in range(2):
                        h = 2 * p + i
                        nc.tensor.matmul(
                            y_ps[i][:, c0:TCH],
                            vaug[:, (h * NBLK + Tt) * VW:
                                 (h * NBLK + Tt + 1) * VW],
                            e2[:, i * 512 + c0:(i + 1) * 512],
                            start=(Tt == 0), stop=(Tt == ntk - 1),
                            skip_group_check=True)
                yp = ypool_sb.tile([128, TCH], F16, tag=f"yp{p}")
                for i in range(2):
                    # denominator sits replicated in psum rows 64:128;
                    # custom-DVE recip misreads PSUM, stage through SBUF
                    dcp = ypool_sb.tile([64, TCH], F32, tag="dcp")
                    nc.vector.tensor_copy(dcp[:, :], y_ps[i][64:128, :])
                    rbc = ypool_sb.tile([64, TCH], F32, tag="rbc")
                    nc.vector.reciprocal_approx_fast(rbc[:, :], dcp[:, :])
                    nc.vector.tensor_mul(yp[i * 64:(i + 1) * 64, :],
                                         y_ps[i][0:64, :], rbc[:, :])
                return yp

            def start_gather(j, yps):
                """Store this core's y, AllGather it, and kick off the
                (collective-dependent) load of the gathered y; returns the
                SBUF tile the load fills."""
                for p in range(NPAIR):
                    nc.sync.dma_start(
                        out=cc_in[j][p * 128:(p + 1) * 128, :],
                        in_=yps[p][:, :])
                nc.gpsimd.collective_compute(
                    "AllGather", mybir.AluOpType.bypass,
                    replica_groups=groups,
                    ins=[cc_in[j].ap().opt()],
                    outs=[cc_out[j].ap().opt()])
                yfull = ypool_sb.tile([128, 8 * TCH], F16, tag="yfull")
                nc.gpsimd.dma_start(
                    out=yfull[:, :].rearrange("p (k t) -> p k t", k=8),
                    in_=cc_out[j].ap().opt().rearrange(
                        "(k p t) -> p k t", p=128, t=TCH))
                return yfull

            def mk_proj(j, yfull):
                def emit():
                    y3 = yfull[:, :].rearrange("p (k t) -> p k t", k=8)
                    for o in range(2):
                        op_ps = mmpool.tile([128, TCH], F32, tag="mm")
                        for k in range(8):
                            nc.tensor.matmul(
                                op_ps[:, :],
                                wp_sb[k][:, o * 128:(o + 1) * 128],
                                y3[:, k, :],
                                start=(k == 0), stop=(k == 7))
                        op_sb = tpool.tile([128, TCH], F16, tag="osb")
                        nc.vector.tensor_copy(op_sb[:, :], op_ps[:, :])
                        nc.sync.dma_start(
                            out=out_ext[j, o * 128:(o + 1) * 128, :],
                            in_=op_sb[:, :])
                return emit

            def start_gather_half(j, p, yp):
                nc.sync.dma_start(out=cc_half_in[j][p][:, :], in_=yp[:, :])
                nc.gpsimd.collective_compute(
                    "AllGather", mybir.AluOpType.bypass,
                    replica_groups=groups,
                    ins=[cc_half_in[j][p].ap().opt()],
                    outs=[cc_half_out[j][p].ap().opt()])
                yh = ypool_sb.tile([128, 4 * TCH], F16, tag=f"yhalf{p}",
                                   name=f"yhalf{p}")
                nc.gpsimd.dma_start(
                    out=yh[:, :].rearrange("p (k t) -> p k t", k=4),
                    in_=cc_half_out[j][p].ap().opt().rearrange(
                        "(k p t) -> p k t", p=128, t=TCH))
                return yh

            gorder = [0, 3, 2, 1]
            for gi, g in enumerate(gorder):
                if gi < NTC - 1:
                    yps = [attn_set(g, p) for p in range(NPAIR)]
                    yfull = start_gather(g, yps)
                    pending[0] = mk_proj(g, yfull)
                else:
                    # final group: gather each pair-half as soon as its
                    # attention set completes, so pair-0's AllGather hides
                    # under pair-1's attention
                    yh = []
                    for p in range(NPAIR):
                        yp = attn_set(g, p)
                        yh.append(start_gather_half(g, p, yp))
                    y3h = [y[:, :].rearrange("p (k t) -> p k t", k=4)
                           for y in yh]
                    for o in range(2):
                        op_ps = mmpool.tile([128, TCH], F32, tag="mm")
                        for k in range(8):
                            hp, i = k % 2, k // 2
                            nc.tensor.matmul(
                                op_ps[:, :],
                                wp_sb[2 * i + hp][:, o * 128:(o + 1) * 128],
                                y3h[hp][:, i, :],
                                start=(k == 0), stop=(k == 7))
                        op_sb = tpool.tile([128, TCH], F16, tag="osb")
                        nc.vector.tensor_copy(op_sb[:, :], op_ps[:, :])
                        nc.sync.dma_start(
                            out=out_ext[g, o * 128:(o + 1) * 128, :],
                            in_=op_sb[:, :])
    if not nc.is_finalized():
        nc.finalize()
    return nc


_NC_CACHE = None


def _get_nc():
    global _NC_CACHE
    if _NC_CACHE is None:
        _NC_CACHE = build()
    return _NC_CACHE


def make_in_maps(x, w_qkv, w_proj):
    x = np.asarray(x, np.float32)
    w_qkv = np.asarray(w_qkv, np.float32)
    w_proj = np.asarray(w_proj, np.float32)
    x_tb = [np.ascontiguousarray(x[b].T).astype(np.float16) for b in range(B)]
    in_maps = []
    for r in range(NCORE):
        b, i = r // GRP, r % GRP
        blocks = []
        for p in range(NPAIR):
            ha, hb = 4 * i + 2 * p, 4 * i + 2 * p + 1
            prows = (list(range(ha * 64, ha * 64 + 64))
                     + list(range(hb * 64, hb * 64 + 64)))
            for m in range(2):  # q, k per pair
                blocks.extend([m * C + q for q in prows])
        # v columns for all 4 heads, in head order
        blocks.extend([2 * C + q
                       for q in range(4 * i * 64, 4 * (i + 1) * 64)])
        w_all = np.ascontiguousarray(w_qkv[blocks, :].T).astype(np.float16)
        orows = list(range(256 * i, 256 * (i + 1)))
        w_p = np.ascontiguousarray(w_proj[orows, :].T).astype(np.float16)
        in_maps.append({"x_t": x_tb[b], "w_all": w_all, "w_p": w_p})
    return in_maps


def assemble(results):
    outT = np.zeros((B, C, T), np.float32)
    for r in range(NCORE):
        b, i = r // GRP, r % GRP
        o = results[r]["out"].astype(np.float32)
        for g in range(NTC):
            outT[b, 256 * i:256 * (i + 1), g * TCH:(g + 1) * TCH] = o[g]
    return np.ascontiguousarray(outT.transpose(0, 2, 1))


def run(x, w_qkv, w_proj, trace=False):
    nc = _get_nc()
    in_maps = make_in_maps(x, w_qkv, w_proj)
    res = run_bass_kernel_spmd(nc, in_maps, list(range(NCORE)), trace=trace)
    return assemble(res.results), res


def kernel(x, w_qkv, w_proj):
    out, _ = run(x, w_qkv, w_proj, trace=False)
    return out


# revision 18
# speedup vs baseline: 1.1148x; 1.1148x over previous
"""Distributed causal attention w/ RoPE for TRN2 (8 NeuronCores).

Sharding: 2-way data-parallel over batch x 4-way tensor-parallel over
heads. Cores {0..3} handle batch 0, {4..7} batch 1; core rank i within
its group owns heads 4i..4i+3 (as two head-pairs packed into 128
partitions). Per core:
  - QKV projection of its batch's full sequence for its 4 heads, in
    transposed layout (qkv^T = W^T.T @ x^T); V is instead projected with
    x as the stationary operand (out[t,d] = x_chunk^T @ w_v), which
    yields V directly in [tk, d] layout -- no on-device V transpose.
  - RoPE via pair-swapped duplicate values (strided SBUF DMA) +
    elementwise DVE ops.
  - Causal attention per (query-group, head-pair): scores for both heads
    of a pair go into one 2-bank PSUM tile so a single merged Exp
    instruction covers them; AV matmuls are software-pipelined one tile
    ahead of the scores stream to keep the PE p-state high. V tiles carry
    64 extra all-ones columns so the AV matmul replicates the softmax
    denominator into PSUM rows 64:128 (a free partition-broadcast on the
    PE); normalization is recip+mul on DVE only.
  - Output projection partials reduced with one ReduceScatter per
    query-group across the 4-core batch group (the two groups' collectives
    are independent); each proj is emitted deferred, two attention steps
    into the next group, so the PE never waits on the normalize chain.
Host side: input layout prep and concatenation of ReduceScatter shards.
"""

import numpy as np

import concourse.bass as bass
import concourse.bacc as bacc
import concourse.mybir as mybir
from concourse import tile
from concourse.bass_utils import run_bass_kernel_spmd

B, T, C, H, D = 2, 2048, 1024, 16, 64
NCORE = 8
GRP = 4                   # cores per batch group
HPC = H // GRP            # heads per core = 4
NPAIR = HPC // 2          # head-pairs per core = 2
TCH = 512                 # token chunk (qkv proj free dim & query group)
NTC = T // TCH            # 4
NBLK = T // 128           # 16 tk tiles
VW = 128                  # vaug block: 64 V cols + 64 ones cols
ROPE_BASE = 10000.0
F32 = mybir.dt.float32
F16 = mybir.dt.float16


def _rope_tables():
    # row p of a pair-tile holds head_local = p // 64, d = p % 64
    d = np.arange(D)
    j = d // 2
    theta = ROPE_BASE ** (-(2.0 * j) / D)          # per-row theta
    t = np.arange(T, dtype=np.float64)
    ang = t[None, :] * theta[:, None]              # [64, T]
    cos = np.cos(ang)
    sin = np.sin(ang)
    sgn = np.where(d % 2 == 0, -1.0, 1.0)[:, None]
    c1 = np.concatenate([cos, cos], axis=0)        # [128, T]
    s1 = np.concatenate([sgn * sin, sgn * sin], axis=0)
    scale = 1.0 / np.sqrt(D)
    return (
        (c1 * scale).astype(np.float16),
        (s1 * scale).astype(np.float16),
        c1.astype(np.float16),
        s1.astype(np.float16),
    )


def build():
    nc = bacc.Bacc(num_devices=NCORE)
    x_t = nc.declare_dram_parameter("x_t", [C, T], F16, isOutput=False)
    w_all = nc.declare_dram_parameter("w_all", [C, 768], F16, isOutput=False)
    w_p = nc.declare_dram_parameter("w_p", [C, 256], F16, isOutput=False)
    out_ext = nc.declare_dram_parameter("out", [NTC, 256, TCH], F16,
                                        isOutput=True)

    cq_np, sq_np, ck_np, sk_np = _rope_tables()
    tk = np.arange(128)[:, None]
    tq = np.arange(128)[None, :]
    mask_np = (tq >= tk).astype(np.float16)
    cq_c = nc.inline_tensor(cq_np, name="cq")
    sq_c = nc.inline_tensor(sq_np, name="sq")
    ck_c = nc.inline_tensor(ck_np, name="ck")
    sk_c = nc.inline_tensor(sk_np, name="sk")
    mask_c = nc.inline_tensor(mask_np, name="mask")

    cc_in = [nc.dram_tensor(f"cc_in{j}", [256, TCH], F16) for j in range(NTC)]
    cc_out = [nc.dram_tensor(f"cc_out{j}", [C, TCH], F16)
              for j in range(NTC)]
    cc_half_in = [[nc.dram_tensor(f"cc_hin{j}_{p}", [128, TCH], F16)
                   for p in range(NPAIR)] for j in range(NTC)]
    cc_half_out = [[nc.dram_tensor(f"cc_hout{j}_{p}", [512, TCH], F16)
                    for p in range(NPAIR)] for j in range(NTC)]
    cc_win = nc.dram_tensor("cc_win", [1, 128], F16)
    cc_wout = nc.dram_tensor("cc_wout", [GRP, 128], F16)
    groups = [[0, 1, 2, 3], [4, 5, 6, 7]]

    with tile.TileContext(nc) as tc:
        with (
            tc.tile_pool(name="const", bufs=1) as cpool,
            tc.tile_pool(name="big", bufs=1) as bpool,
            tc.tile_pool(name="xt", bufs=16) as xpool,
            tc.tile_pool(name="tmp", bufs=4) as tpool,
            tc.tile_pool(name="exp", bufs=4) as epool,
            tc.tile_pool(name="ysmall", bufs=2) as ypool_sb,
            tc.tile_pool(name="s2", bufs=2, space="PSUM") as s2pool,
            tc.tile_pool(name="ypsum", bufs=2, space="PSUM") as ypool,
            tc.tile_pool(name="mm", bufs=2, space="PSUM") as mmpool,
        ):
            # ---- persistent SBUF loads (weights first: unblock matmuls) ----
            w_sb = []
            for c in range(8):
                w = cpool.tile([128, 768], F16, tag=f"w{c}")
                nc.scalar.dma_start(out=w[:, :],
                                    in_=w_all[c * 128:(c + 1) * 128, :])
                w_sb.append(w)
            wp_sb = []
            for k in range(8):
                wp = cpool.tile([128, 256], F16, tag=f"wp{k}")
                nc.scalar.dma_start(out=wp[:, :],
                                    in_=w_p[k * 128:(k + 1) * 128, :])
                wp_sb.append(wp)
            cq_sb = cpool.tile([128, T], F16, tag="cq")
            nc.scalar.dma_start(out=cq_sb[:, :], in_=cq_c[:, :])
            sq_sb = cpool.tile([128, T], F16, tag="sq")
            nc.scalar.dma_start(out=sq_sb[:, :], in_=sq_c[:, :])
            ck_sb = cpool.tile([128, T], F16, tag="ck")
            nc.scalar.dma_start(out=ck_sb[:, :], in_=ck_c[:, :])
            sk_sb = cpool.tile([128, T], F16, tag="sk")
            nc.scalar.dma_start(out=sk_sb[:, :], in_=sk_c[:, :])
            mask_sb = cpool.tile([128, 128], F16, tag="mask")
            nc.scalar.dma_start(out=mask_sb[:, :], in_=mask_c[:, :])

            # warmup collective to absorb one-time mesh/channel setup
            nc.gpsimd.collective_compute(
                "AllGather", mybir.AluOpType.bypass,
                replica_groups=groups,
                ins=[cc_win.ap().opt()],
                outs=[cc_wout.ap().opt()])

            rope_q = [bpool.tile([128, T], F16, tag=f"rope_q{p}",
                                 name=f"rope_q{p}") for p in range(NPAIR)]
            rope_k = [bpool.tile([128, T], F16, tag=f"rope_k{p}",
                                 name=f"rope_k{p}") for p in range(NPAIR)]
            # one [tk, d|ones] V tile per head, packed in a single buffer:
            # head h occupies cols [h*NBLK*VW, (h+1)*NBLK*VW)
            vaug = bpool.tile([128, HPC * NBLK * VW], F16, tag="vaug")
            nc.vector.memset(vaug[:, :], 1.0)
            va3 = vaug[:, :].rearrange("p (h b) -> p h b", h=HPC)

            # ---- QKV projection + fused RoPE ------------------------------
            for tc_i in range(NTC):
                t0 = tc_i * TCH
                xts = []
                for c in range(8):
                    xt = xpool.tile([128, TCH], F16, tag="xt")
                    nc.sync.dma_start(
                        out=xt[:, :],
                        in_=x_t[c * 128:(c + 1) * 128, t0:t0 + TCH])
                    xts.append(xt)
                for p in range(NPAIR):
                    qk_ps = s2pool.tile([128, 1024], F32, tag="s2")
                    for m in range(2):  # q, k
                        for c in range(8):
                            nc.tensor.matmul(
                                qk_ps[:, m * 512:(m + 1) * 512],
                                w_sb[c][:, p * 256 + m * 128:
                                        p * 256 + (m + 1) * 128],
                                xts[c][:, :],
                                start=(c == 0), stop=(c == 7))
                    qk_sb = tpool.tile([128, 1024], F16, tag="qksb")
                    nc.vector.tensor_copy(qk_sb[:, :], qk_ps[:, :])
                    qks_sw = tpool.tile([128, 1024], F16, tag="qkssb")
                    nc.sync.dma_start(out=qks_sw[0::2, :], in_=qk_sb[1::2, :])
                    nc.sync.dma_start(out=qks_sw[1::2, :], in_=qk_sb[0::2, :])
                    # rope_q = q*cq + qs*sq ; rope_k = k*ck + ks*sk
                    tq1 = tpool.tile([128, TCH], F16, tag="tmp")
                    nc.vector.tensor_mul(tq1[:, :], qk_sb[:, 0:512],
                                         cq_sb[:, t0:t0 + TCH])
                    tq2 = tpool.tile([128, TCH], F16, tag="tmp")
                    nc.vector.tensor_mul(tq2[:, :], qks_sw[:, 0:512],
                                         sq_sb[:, t0:t0 + TCH])
                    nc.vector.tensor_add(rope_q[p][:, t0:t0 + TCH], tq1[:, :],
                                         tq2[:, :])
                    tk1 = tpool.tile([128, TCH], F16, tag="tmp")
                    nc.vector.tensor_mul(tk1[:, :], qk_sb[:, 512:1024],
                                         ck_sb[:, t0:t0 + TCH])
                    tk2 = tpool.tile([128, TCH], F16, tag="tmp")
                    nc.vector.tensor_mul(tk2[:, :], qks_sw[:, 512:1024],
                                         sk_sb[:, t0:t0 + TCH])
                    nc.vector.tensor_add(rope_k[p][:, t0:t0 + TCH], tk1[:, :],
                                         tk2[:, :])
                # V in [tk, d] layout directly: x chunks stationary
                for sub in range(4):
                    Tt = 4 * tc_i + sub
                    vt_ps = mmpool.tile([128, TCH], F32, tag="mm")
                    for c in range(8):
                        nc.tensor.matmul(
                            vt_ps[:, 0:256],
                            xts[c][:, sub * 128:(sub + 1) * 128],
                            w_sb[c][:, 512:768],
                            start=(c == 0), stop=(c == 7))
                    nc.vector.tensor_copy(
                        va3[:, :, Tt * VW:Tt * VW + 64],
                        vt_ps[:, 0:256].rearrange("p (h d) -> p h d", d=64))

            # ---- attention + deferred output projection -------------------
            pending = [None]

            def attn_set(g, p, flush_at=15):
                """Attention for query group g, head pair p; returns the
                normalized [128, TCH] fp16 y tile (rows = 2 heads x 64)."""
                ntk = 4 * g + 4
                base = p * ntk
                q0 = g * TCH

                def emit_scores(Tt):
                    diag = (Tt // 4 == g)
                    r = Tt % 4
                    c0 = 128 * r if diag else 0
                    s2 = s2pool.tile([128, 1024], F32, tag="s2")
                    for i in range(2):
                        nc.tensor.matmul(
                            s2[:, i * 512 + c0:(i + 1) * 512],
                            rope_k[p][i * 64:(i + 1) * 64,
                                      Tt * 128:(Tt + 1) * 128],
                            rope_q[p][i * 64:(i + 1) * 64,
                                      q0 + c0:q0 + TCH],
                            start=True, stop=True)
                    e2 = epool.tile([128, 1024], F16, tag="e2")
                    if diag:
                        for i in range(2):
                            nc.scalar.activation(
                                e2[:, i * 512 + c0:(i + 1) * 512],
                                s2[:, i * 512 + c0:(i + 1) * 512],
                                mybir.ActivationFunctionType.Exp)
                            nc.vector.tensor_mul(
                                e2[:, i * 512 + c0:i * 512 + c0 + 128],
                                e2[:, i * 512 + c0:i * 512 + c0 + 128],
                                mask_sb[:, :])
                    else:
                        nc.scalar.activation(
                            e2[:, :], s2[:, :],
                            mybir.ActivationFunctionType.Exp)
                    return e2, c0

                y_ps = [ypool.tile([128, TCH], F32, tag="y", name=f"y{i}")
                        for i in range(2)]
                cur = emit_scores(0)
                for Tt in range(ntk):
                    e2, c0 = cur
                    if Tt + 1 < ntk:
                        cur = emit_scores(Tt + 1)
                    if base + Tt >= flush_at and pending[0] is not None:
                        pending[0]()
                        pending[0] = None
                    for i in range(2):
                        h = 2 * p + i
                        nc.tensor.matmul(
                            y_ps[i][:, c0:TCH],
                            vaug[:, (h * NBLK + Tt) * VW:
                                 (h * NBLK + Tt + 1) * VW],
                            e2[:, i * 512 + c0:(i + 1) * 512],
                            start=(Tt == 0), stop=(Tt == ntk - 1),
                            skip_group_check=True)
                yp = ypool_sb.tile([128, TCH], F16, tag=f"yp{p}")
                for i in range(2):
                    # denominator sits replicated in psum rows 64:128;
                    # custom-DVE recip misreads PSUM, stage through SBUF
                    dcp = ypool_sb.tile([64, TCH], F32, tag="dcp")
                    nc.vector.tensor_copy(dcp[:, :], y_ps[i][64:128, :])
                    rbc = ypool_sb.tile([64, TCH], F32, tag="rbc")
                    nc.vector.reciprocal_approx_fast(rbc[:, :], dcp[:, :])
                    nc.vector.tensor_mul(yp[i * 64:(i + 1) * 64, :],
                                         y_ps[i][0:64, :], rbc[:, :])
                return yp

            def start_gather(j, yps):
                """Store this core's y, AllGather it, and kick off the
                (collective-dependent) load of the gathered y; returns the
                SBUF tile the load fills."""
                for p in range(NPAIR):
                    nc.sync.dma_start(
                        out=cc_in[j][p * 128:(p + 1) * 128, :],
                        in_=yps[p][:, :])
                nc.gpsimd.collective_compute(
                    "AllGather", mybir.AluOpType.bypass,
                    replica_groups=groups,
                    ins=[cc_in[j].ap().opt()],
                    outs=[cc_out[j].ap().opt()])
                yfull = ypool_sb.tile([128, 8 * TCH], F16, tag="yfull")
                nc.gpsimd.dma_start(
                    out=yfull[:, :].rearrange("p (k t) -> p k t", k=8),
                    in_=cc_out[j].ap().opt().rearrange(
                        "(k p t) -> p k t", p=128, t=TCH))
                return yfull

            def mk_proj(j, yfull):
                def emit():
                    y3 = yfull[:, :].rearrange("p (k t) -> p k t", k=8)
                    for o in range(2):
                        op_ps = mmpool.tile([128, TCH], F32, tag="mm")
                        for k in range(8):
                            nc.tensor.matmul(
                                op_ps[:, :],
                                wp_sb[k][:, o * 128:(o + 1) * 128],
                                y3[:, k, :],
                                start=(k == 0), stop=(k == 7))
                        op_sb = tpool.tile([128, TCH], F16, tag="osb")
                        nc.vector.tensor_copy(op_sb[:, :], op_ps[:, :])
                        nc.sync.dma_start(
                            out=out_ext[j, o * 128:(o + 1) * 128, :],
                            in_=op_sb[:, :])
                return emit

            def start_gather_half(j, p, yp):
                nc.sync.dma_start(out=cc_half_in[j][p][:, :], in_=yp[:, :])
                nc.gpsimd.collective_compute(
                    "AllGather", mybir.AluOpType.bypass,
                    replica_groups=groups,
                    ins=[cc_half_in[j][p].ap().opt()],
                    outs=[cc_half_out[j][p].ap().opt()])
                yh = ypool_sb.tile([128, 4 * TCH], F16, tag=f"yhalf{p}",
                                   name=f"yhalf{p}")
                nc.gpsimd.dma_start(
                    out=yh[:, :].rearrange("p (k t) -> p k t", k=4),
                    in_=cc_half_out[j][p].ap().opt().rearrange(
                        "(k p t) -> p k t", p=128, t=TCH))
                return yh

            gorder = [0, 3, 2, 1]
            for gi, g in enumerate(gorder):
                fl = min(15, 2 * (4 * g + 4) - 3)
                if gi < NTC - 1:
                    yps = [attn_set(g, p, flush_at=fl) for p in range(NPAIR)]
                    yfull = start_gather(g, yps)
                    pending[0] = mk_proj(g, yfull)
                else:
                    # final group: gather each pair-half as soon as its
                    # attention set completes, so pair-0's AllGather hides
                    # under pair-1's attention
                    yh = []
                    for p in range(NPAIR):
                        yp = attn_set(g, p, flush_at=fl)
                        yh.append(start_gather_half(g, p, yp))
                    y3h = [y[:, :].rearrange("p (k t) -> p k t", k=4)
                           for y in yh]
                    for o in range(2):
                        op_ps = mmpool.tile([128, TCH], F32, tag="mm")
                        for k in range(8):
                            hp, i = k // 4, k % 4
                            nc.tensor.matmul(
                                op_ps[:, :],
                                wp_sb[2 * i + hp][:, o * 128:(o + 1) * 128],
                                y3h[hp][:, i, :],
                                start=(k == 0), stop=(k == 7))
                        op_sb = tpool.tile([128, TCH], F16, tag="osb")
                        nc.vector.tensor_copy(op_sb[:, :], op_ps[:, :])
                        nc.sync.dma_start(
                            out=out_ext[g, o * 128:(o + 1) * 128, :],
                            in_=op_sb[:, :])
    if not nc.is_finalized():
        nc.finalize()
    return nc


_NC_CACHE = None


def _get_nc():
    global _NC_CACHE
    if _NC_CACHE is None:
        _NC_CACHE = build()
    return _NC_CACHE


def make_in_maps(x, w_qkv, w_proj):
    x = np.asarray(x, np.float32)
    w_qkv = np.asarray(w_qkv, np.float32)
    w_proj = np.asarray(w_proj, np.float32)
    x_tb = [np.ascontiguousarray(x[b].T).astype(np.float16) for b in range(B)]
    in_maps = []
    for r in range(NCORE):
        b, i = r // GRP, r % GRP
        blocks = []
        for p in range(NPAIR):
            ha, hb = 4 * i + 2 * p, 4 * i + 2 * p + 1
            prows = (list(range(ha * 64, ha * 64 + 64))
                     + list(range(hb * 64, hb * 64 + 64)))
            for m in range(2):  # q, k per pair
                blocks.extend([m * C + q for q in prows])
        # v columns for all 4 heads, in head order
        blocks.extend([2 * C + q
                       for q in range(4 * i * 64, 4 * (i + 1) * 64)])
        w_all = np.ascontiguousarray(w_qkv[blocks, :].T).astype(np.float16)
        orows = list(range(256 * i, 256 * (i + 1)))
        w_p = np.ascontiguousarray(w_proj[orows, :].T).astype(np.float16)
        in_maps.append({"x_t": x_tb[b], "w_all": w_all, "w_p": w_p})
    return in_maps


def assemble(results):
    outT = np.zeros((B, C, T), np.float32)
    for r in range(NCORE):
        b, i = r // GRP, r % GRP
        o = results[r]["out"].astype(np.float32)
        for g in range(NTC):
            outT[b, 256 * i:256 * (i + 1), g * TCH:(g + 1) * TCH] = o[g]
    return np.ascontiguousarray(outT.transpose(0, 2, 1))


def run(x, w_qkv, w_proj, trace=False):
    nc = _get_nc()
    in_maps = make_in_maps(x, w_qkv, w_proj)
    res = run_bass_kernel_spmd(nc, in_maps, list(range(NCORE)), trace=trace)
    return assemble(res.results), res


def kernel(x, w_qkv, w_proj):
    out, _ = run(x, w_qkv, w_proj, trace=False)
    return out
